# revision 20
# baseline (speedup 1.0000x reference)
"""2-layer dense GCN on 8 Trainium2 NeuronCores — fp8 residual + DoubleRow.

Reference computation (all fp32):
    H0 = relu((A_norm @ X) @ W0)
    H1 = relu((A_norm @ H0) @ W1)
A_norm: [16384, 16384] row-stochastic, X: [16384, 128], W0/W1: [128, 128].

Scheme: A_norm rows sum to exactly 1, so A = (1/N)*ones + R with R zero-mean
uniform. Only the residual is streamed, as e4m3 scaled to full range; the
rank-1 part is an exact per-feature bias (s*mu * colsum(H), with the TRUE
colsum for X host-side; for layer 1 the colsum is computed on-device from the
gathered e5m2 H via 32 width-512 ones-matmuls into a [1,512] PSUM bank — its
quantization noise averages down over 16384 nodes). Stationary X/H live in
e5m2. Aggregation runs in DoubleRow mode (2 contraction subtiles per PE pass).

Device structure (per core, 2048 output rows, 1D row shard):
  - layer 0 chunk-major: one 512-row output chunk per full-contraction pass;
    eviction: mt = (psum + bias)*(1/s) fp16; H = relu(mt.T @ W) written
    straight to e5m2; one SWDGE bounce write + AllGather doorbell per chunk.
  - exchange: FOUR chunked AllGathers (64 KiB e5m2 payloads). All stat1
    gather loads ride the gpsimd queue right behind their doorbell, so each
    fires the instant its AllGather lands and the HWDGE rings stay pure
    A-stream.
  - layer 1 quarter-major: pass q sweeps all 4 output chunks against
    stationary quarter q (gated only on AllGather q). The q3 A-block column
    plus (c2,q2),(c3,q2) stay RESIDENT in SBUF from layer 0 (12 MiB cache),
    so the post-AG3 tail needs no A DMA at all: stat1-q3 load + 64 DR
    matmuls + evictions (~30 us). Layer 1 streams only 10 of 16 blocks.

DMA routing: the A stream (26 blocks x 2 MiB) owns the two HWDGE rings
(sync/scalar) end to end; stationary X is ONE 2 MiB HWDGE load; everything
collective-gated is SWDGE so AllGather waits never block the A stream.

Dispatch: inputs are pre-staged onto all 8 cores (device_put + block) before
the single execution, so per-core start skew is not inflated by operand
transfer ordering.
"""

import sys
from contextlib import ExitStack

if "/opt/trn_rl_repo" not in sys.path:
    sys.path.insert(0, "/opt/trn_rl_repo")

import numpy as np

N_NODES = 16384
D = 128
NCORES = 8
ROWS = N_NODES // NCORES      # 2048
NCH = 4                       # output chunks per core (512 rows each)
IC = ROWS // NCH              # 512
NQ = 4                        # contraction quarters (= exchange chunks)
NT = 4                        # j-subtiles per (quarter, peer)

# layer-1 A-blocks resident in SBUF from layer 0: the whole q3 column, so
# the post-AG3 tail needs no A DMA at all. The rest of SBUF goes to stream
# depth (a_pool) — a deeper pool decouples the layer-1 prefetch from
# layer 0's PE-paced consumption (buffer recycling stalls).
CACHED = {(0, 3), (1, 3), (2, 3), (3, 3)}

PRECISION = "dr"  # tag for test.py compatibility
DEBUG = False     # adds intermediate dumps (d_h0, d_sig, d_bias) as outputs


def build_gcn():
    import concourse.bass as bass  # noqa: F401
    import concourse.tile as tile
    from concourse import bacc, mybir

    F32 = mybir.dt.float32
    F16 = mybir.dt.float16
    E4 = mybir.dt.float8e4
    E5 = mybir.dt.float8e5
    DR = mybir.MatmulPerfMode.DoubleRow
    relu = mybir.ActivationFunctionType.Relu
    add = mybir.AluOpType.add
    mult = mybir.AluOpType.mult

    nc = bacc.Bacc("TRN2", target_bir_lowering=False, num_devices=NCORES)

    # A residual, host pre-tiled into 2 MiB blocks (16 KiB partition lines):
    # block (c, q) is rows [(c*4+q)*128, +128); element (p, (r*4+t)*512 + cc)
    #   = s * R[myrows0 + c*512 + cc, r*2048 + q*512 + t*128 + p]
    a_in = nc.dram_tensor(
        "a0", [NCH * NQ * 128, NCORES * NT * IC], E4, kind="ExternalInput"
    )
    # X stationary as ONE contiguous [128, 16384] block:
    # x0[p, (r*16 + q*4 + t)*128 + dd] = X[r*2048 + q*512 + t*128 + p, dd]
    x_in = nc.dram_tensor("x0", [128, NCORES * NQ * NT * D], E5, kind="ExternalInput")
    w0 = nc.dram_tensor("w0", [D, D], F16, kind="ExternalInput")
    w1 = nc.dram_tensor("w1", [D, D], F16, kind="ExternalInput")
    b0 = nc.dram_tensor("b0", [D, 1], F32, kind="ExternalInput")   # s*mu*colsum(X)
    smu = nc.dram_tensor("smu", [D, 1], F32, kind="ExternalInput")  # s*mu
    is_in = nc.dram_tensor("is_", [D, 1], F32, kind="ExternalInput")  # 1/s
    h_out = nc.dram_tensor("h_out", [ROWS, D], F32, kind="ExternalOutput")
    if DEBUG:
        d_h0 = nc.dram_tensor("d_h0", [128, NCH * IC], E5, kind="ExternalOutput")
        d_sig = nc.dram_tensor("d_sig", [1, IC], F32, kind="ExternalOutput")
        d_bias = nc.dram_tensor("d_bias", [D, 1], F32, kind="ExternalOutput")

    with tile.TileContext(nc) as tc, ExitStack() as ctx:
        sb1 = ctx.enter_context(tc.tile_pool(name="sb1", bufs=1))
        stat0_pool = ctx.enter_context(tc.tile_pool(name="stat0", bufs=1))
        stat1_pool = ctx.enter_context(tc.tile_pool(name="stat1", bufs=NQ))
        a_pool = ctx.enter_context(tc.tile_pool(name="a", bufs=6))
        ac_pool = ctx.enter_context(tc.tile_pool(name="ac", bufs=len(CACHED)))
        m_pool = ctx.enter_context(tc.tile_pool(name="m", bufs=2))
        hc_pool = ctx.enter_context(tc.tile_pool(name="hc", bufs=2))
        h_pool = ctx.enter_context(tc.tile_pool(name="h", bufs=4))
        agg_pool = ctx.enter_context(tc.tile_pool(name="agg", bufs=4, space="PSUM"))
        sig_pool = ctx.enter_context(tc.tile_pool(name="sig", bufs=1, space="PSUM"))
        lin_pool = ctx.enter_context(tc.tile_pool(name="lin", bufs=2, space="PSUM"))
        sgt_pool = ctx.enter_context(tc.tile_pool(name="sgt", bufs=1, space="PSUM"))
        dram = ctx.enter_context(tc.tile_pool(name="dram", bufs=1, space="DRAM"))

        # small constants via SWDGE; stationary X as one HWDGE load on the
        # sync ring while the scalar ring starts the A stream
        w0_sb = sb1.tile([D, D], F16)
        nc.gpsimd.dma_start(out=w0_sb[:], in_=w0[:])
        w1_sb = sb1.tile([D, D], F16)
        nc.gpsimd.dma_start(out=w1_sb[:], in_=w1[:])
        b0_sb = sb1.tile([D, 1], F32)
        nc.gpsimd.dma_start(out=b0_sb[:], in_=b0[:])
        smu_sb = sb1.tile([D, 1], F32)
        nc.gpsimd.dma_start(out=smu_sb[:], in_=smu[:])
        is_sb = sb1.tile([D, 1], F32)
        nc.gpsimd.dma_start(out=is_sb[:], in_=is_in[:])
        ones8 = sb1.tile([D, 1], E5)
        nc.vector.memset(ones8[:], 1.0)
        one1 = sb1.tile([1, 1], F32)
        nc.vector.memset(one1[:], 1.0)

        # stat0 split in halves across both rings, after each ring's first
        # A-half, so neither ring's A stream is displaced by the full 2 MiB
        stat0 = stat0_pool.tile([128, NCORES * NQ * NT, D], E5, name="sx")
        HW = NCORES * NQ * NT // 2  # 64 subtiles = 8 KiB per partition line

        # exchange buffers: one bounce + gathered tensor per chunk
        h_tb = [dram.tile([128, IC], E5, name=f"h_tb{c}") for c in range(NCH)]
        h_ag = [
            dram.tile([NCORES, 128, IC], E5, addr_space="Shared", name=f"h_ag{c}")
            for c in range(NCH)
        ]
        stat1 = [
            stat1_pool.tile([128, NCORES * NT, D], E5, name=f"sh{q}", tag="s1")
            for q in range(NQ)
        ]



        a_cache = {}
        rings = [nc.scalar, nc.sync]
        AH = NCORES * NT // 2  # 16 subtiles = 8 KiB per partition line

        def a_load(c, q):
            """Each block split into peer-halves, one per ring: the block
            lands in ~6 us (ring cadence) instead of ~12, and peers 0-3's
            matmuls can start while peers 4-7 still stream."""
            if (c, q) in CACHED:
                at = ac_pool.tile(
                    [128, NCORES * NT, IC], E4, name=f"ac{c}{q}", tag="ac"
                )
                a_cache[(c, q)] = at
            else:
                at = a_pool.tile([128, NCORES * NT, IC], E4, name="at", tag="at")
            blk = c * NQ + q
            rows = a_in[blk * 128 : (blk + 1) * 128, :]
            rings[0].dma_start(out=at[:, 0:AH, :], in_=rows[:, 0 : AH * IC])
            rings[1].dma_start(out=at[:, AH:, :], in_=rows[:, AH * IC :])
            return at

        def sweep(agg, at, lhsT_fn, first, last):
            """16 DoubleRow matmuls over (peer r, subtile pair tp)."""
            for r in range(NCORES):
                for tp in range(0, NT, 2):
                    nc.tensor.matmul(
                        agg[:],
                        lhsT=lhsT_fn(r, tp),
                        rhs=at[:, r * NT + tp : r * NT + tp + 2, :],
                        start=first and r == 0 and tp == 0,
                        stop=last and r == NCORES - 1 and tp == NT - 2,
                        perf_mode=DR,
                    )

        # ---------------- layer 0 (chunk-major) ----------------
        # first A block ahead of stat0 on both rings: peer-0 matmuls can
        # start ~7 us in, and the c0->AllGather0 chain begins ASAP
        at00 = a_load(0, 0)
        rings[0].dma_start(out=stat0[:, 0:HW, :], in_=x_in[:, 0 : HW * D])
        rings[1].dma_start(out=stat0[:, HW:, :], in_=x_in[:, HW * D :])

        for c in range(NCH):
            agg = agg_pool.tile([128, IC], F32, name="ps", tag="ps")
            for q in range(NQ):
                at = at00 if (c == 0 and q == 0) else a_load(c, q)
                sweep(
                    agg,
                    at,
                    lambda r, tp, q=q: stat0[
                        :, r * NQ * NT + q * NT + tp : r * NQ * NT + q * NT + tp + 2, :
                    ],
                    first=q == 0,
                    last=q == NQ - 1,
                )
            mt = m_pool.tile([128, IC], F16, name="mt", tag="mt")
            nc.vector.tensor_scalar(
                out=mt[:], in0=agg[:], scalar1=b0_sb[:], scalar2=is_sb[:],
                op0=add, op1=mult,
            )
            hc = hc_pool.tile([128, IC], E5, name="hc", tag="hc")
            for t in range(NT):
                lp = lin_pool.tile([128, D], F32, name="lp", tag="lp")
                nc.tensor.matmul(
                    lp[:],
                    lhsT=mt[:, t * 128 : (t + 1) * 128],
                    rhs=w0_sb[:],
                    start=True,
                    stop=True,
                )
                ht = h_pool.tile([128, D], F32, name="ht", tag="ht")
                nc.scalar.activation(ht[:], lp[:], relu)
                nc.vector.tensor_copy(
                    out=hc[:, t * 128 : (t + 1) * 128], in_=ht[:]
                )
            # bounce + doorbell + gather loads, all on the gpsimd queue so the
            # stat1 load for chunk c fires the moment AllGather c completes
            if DEBUG:
                nc.gpsimd.dma_start(out=d_h0[:, c * IC : (c + 1) * IC], in_=hc[:])
            nc.gpsimd.dma_start(out=h_tb[c][:], in_=hc[:])
            nc.gpsimd.collective_compute(
                "AllGather",
                mybir.AluOpType.bypass,
                replica_groups=[list(range(NCORES))],
                ins=[h_tb[c][:]],
                outs=[h_ag[c][:]],
            )
            # single strided load of the whole gathered quarter: [8,128,IC]
            # viewed as [128, 8, IC] — one DMA instead of 8, so the next
            # chunk's doorbell is not stuck behind a long SWDGE queue
            nc.gpsimd.dma_start(
                out=stat1[c][:], in_=h_ag[c][:].transpose([1, 0, 2])
            )

        # ---------------- layer 1 (quarter-major) ----------------
        aggs = [
            agg_pool.tile([128, IC], F32, name=f"ps{c}", tag="ps") for c in range(NCH)
        ]
        sig = sig_pool.tile([1, IC], F32, name="sg")

        def sig_mms(q):
            # colsum of gathered H, one width-512 matmul per peer
            for r in range(NCORES):
                nc.tensor.matmul(
                    sig[:],
                    lhsT=ones8[:],
                    rhs=stat1[q][:, r * NT : (r + 1) * NT, :],
                    start=q == 0 and r == 0,
                    stop=q == NQ - 1 and r == NCORES - 1,
                )

        def evict1(c, bias1):
            mt = m_pool.tile([128, IC], F16, name="mt", tag="mt")
            nc.vector.tensor_scalar(
                out=mt[:], in0=aggs[c][:], scalar1=bias1[:], scalar2=is_sb[:],
                op0=add, op1=mult,
            )
            for t in range(NT):
                lp = lin_pool.tile([128, D], F32, name="lp", tag="lp")
                nc.tensor.matmul(
                    lp[:],
                    lhsT=mt[:, t * 128 : (t + 1) * 128],
                    rhs=w1_sb[:],
                    start=True,
                    stop=True,
                )
                ht = h_pool.tile([128, D], F32, name="ht", tag="ht")
                nc.scalar.activation(ht[:], lp[:], relu)
                eng = rings[(c * NT + t) % 2]
                eng.dma_start(
                    out=h_out[c * IC + t * 128 : c * IC + (t + 1) * 128, :],
                    in_=ht[:],
                )

        bias1 = None
        for q in range(NQ):
            ats = []
            for c in range(NCH):
                if (c, q) in CACHED:
                    ats.append(a_cache[(c, q)])
                else:
                    ats.append(a_load(c, q))
            if q == NQ - 1:
                sig_mms(q)  # before the sweeps: closes the sigma accumulation
            for c in range(NCH):
                sweep(
                    aggs[c],
                    ats[c],
                    lambda r, tp, q=q: stat1[q][:, r * NT + tp : r * NT + tp + 2, :],
                    first=q == 0,
                    last=q == NQ - 1,
                )
                if q == NQ - 1:
                    if c == 0:
                        # sigma -> bias1: reduce 4 subtile groups, transpose
                        # [1,128] -> [128,1] via a 1-partition matmul, * s*mu
                        sigsb = sb1.tile([1, IC], F32, name="sigsb")
                        nc.vector.tensor_copy(out=sigsb[:], in_=sig[:])
                        p01 = sb1.tile([1, D], F32, name="p01")
                        nc.vector.tensor_tensor(
                            out=p01[:], in0=sigsb[:, 0:128], in1=sigsb[:, 128:256],
                            op=add,
                        )
                        p23 = sb1.tile([1, D], F32, name="p23")
                        nc.vector.tensor_tensor(
                            out=p23[:], in0=sigsb[:, 256:384], in1=sigsb[:, 384:512],
                            op=add,
                        )
                        s4 = sb1.tile([1, D], F32, name="s4")
                        nc.vector.tensor_tensor(
                            out=s4[:], in0=p01[:], in1=p23[:], op=add
                        )
                        sgt = sgt_pool.tile([D, 1], F32, name="sgt")
                        nc.tensor.matmul(
                            sgt[:], lhsT=s4[:], rhs=one1[:], start=True, stop=True
                        )
                        bias1 = sb1.tile([D, 1], F32, name="bias1")
                        nc.vector.tensor_tensor(
                            out=bias1[:], in0=sgt[:], in1=smu_sb[:], op=mult
                        )
                        if DEBUG:
                            nc.gpsimd.dma_start(out=d_sig[:], in_=sigsb[:])
                            nc.gpsimd.dma_start(out=d_bias[:], in_=bias1[:])
                    evict1(c, bias1)
            if q < NQ - 1:
                sig_mms(q)  # sigma contributions for this quarter

    nc.finalize()
    return nc


def shard_inputs(A_norm, X, W0, W1, precision=None):
    """Host-side prep. Returns per-core input maps (complete, incl. weights)."""
    import ml_dtypes

    e4m3 = ml_dtypes.float8_e4m3
    e5m2 = ml_dtypes.float8_e5m2
    N = N_NODES
    mu = np.float32(1.0 / N)

    R = A_norm.astype(np.float32) - mu
    s = float(ml_dtypes.finfo(e4m3).max) / float(np.abs(R).max())
    Rq = (R * np.float32(s)).astype(e4m3)
    del R

    # x0[p, (r*16 + q*4 + t)*128 + dd] = X[r*2048 + (q*4+t)*128 + p, dd]
    x0 = np.ascontiguousarray(
        X.astype(e5m2)
        .reshape(NCORES, NQ * NT, 128, D)
        .transpose(2, 0, 1, 3)
        .reshape(128, NCORES * NQ * NT * D)
    )

    smu_v = np.float32(s * mu)
    smu = np.full((D, 1), smu_v, dtype=np.float32)
    # TRUE colsum of X (float64) — kills the rank-1 projection of X's
    # quantization noise
    b0 = (
        np.float64(s) * np.float64(mu) * X.astype(np.float64).sum(axis=0)
    ).astype(np.float32).reshape(D, 1)
    is_ = np.full((D, 1), np.float32(1.0) / np.float32(s), dtype=np.float32)
    w0 = W0.astype(np.float16)
    w1 = W1.astype(np.float16)

    in_maps = []
    for core in range(NCORES):
        Rt = Rq[core * ROWS : (core + 1) * ROWS, :].T  # [16384 nodes, 2048]
        # [r, q, t, p, c, cc] -> rows (c*4+q)*128+p, cols (r*4+t)*512+cc
        a0 = np.ascontiguousarray(
            Rt.reshape(NCORES, NQ, NT, 128, NCH, IC)
            .transpose(4, 1, 3, 0, 2, 5)
            .reshape(NCH * NQ * 128, NCORES * NT * IC)
        )
        in_maps.append(
            {"a0": a0, "x0": x0, "w0": w0, "w1": w1, "b0": b0, "smu": smu,
             "is_": is_}
        )
    return in_maps


_CACHED = {}
_EXEC = {}


def _exec_staged(nc, in_maps):
    """Run the prebuilt module via PJRT with inputs pre-staged (device_put +
    block_until_ready) so all 8 cores launch with operands already resident —
    avoids per-core start skew from operand transfer ordering."""
    import jax
    from jax.experimental.shard_map import shard_map
    from jax.sharding import Mesh, NamedSharding, PartitionSpec

    from concourse import bass2jax, mybir

    key = id(nc)
    if key not in _EXEC:
        bass2jax.install_neuronx_cc_hook()
        partition_name = (
            nc.partition_id_tensor.name if nc.partition_id_tensor else None
        )
        in_names, out_names, out_avals = [], [], []
        for alloc in nc.m.functions[0].allocations:
            if not isinstance(alloc, mybir.MemoryLocationSet):
                continue
            name = alloc.memorylocations[0].name
            if alloc.kind == "ExternalInput":
                if name != partition_name:
                    in_names.append(name)
            elif alloc.kind == "ExternalOutput":
                out_names.append(name)
                shape = tuple(alloc.tensor_shape)
                dtype = mybir.dt.np(alloc.dtype)
                out_avals.append(jax.core.ShapedArray(shape, dtype))
        n_params = len(in_names)
        all_in = list(in_names) + list(out_names)
        if partition_name is not None:
            all_in.append(partition_name)

        def _body(*args):
            operands = list(args)
            if partition_name is not None:
                operands.append(bass2jax.partition_id_tensor())
            outs = bass2jax._bass_exec_p.bind(
                *operands,
                out_avals=tuple(out_avals),
                in_names=tuple(all_in),
                out_names=tuple(out_names),
                lowering_input_output_aliases=(),
                sim_require_finite=True,
                sim_require_nnan=True,
                nc=nc,
            )
            return tuple(outs)

        devices = jax.devices()[:NCORES]
        mesh = Mesh(np.asarray(devices), ("core",))
        n_outs = len(out_avals)
        sharded = jax.jit(
            shard_map(
                _body,
                mesh=mesh,
                in_specs=(PartitionSpec("core"),) * (n_params + n_outs),
                out_specs=(PartitionSpec("core"),) * n_outs,
                check_rep=False,
            ),
            donate_argnums=tuple(range(n_params, n_params + n_outs)),
            keep_unused=True,
        )
        _EXEC[key] = (sharded, in_names, out_names, out_avals, mesh)
    sharded, in_names, out_names, out_avals, mesh = _EXEC[key]

    sh = NamedSharding(mesh, PartitionSpec("core"))
    concat_in = [
        np.concatenate([np.asarray(in_maps[c][nm]) for c in range(NCORES)], axis=0)
        for nm in in_names
    ]
    concat_zeros = [
        np.zeros((NCORES * a.shape[0], *a.shape[1:]), a.dtype) for a in out_avals
    ]
    import jax

    staged = [jax.device_put(a, sh) for a in concat_in + concat_zeros]
    for a in staged:
        a.block_until_ready()
    out_arrs = sharded(*staged)
    return [
        {
            nm: np.asarray(out_arrs[i]).reshape(NCORES, *out_avals[i].shape)[c]
            for i, nm in enumerate(out_names)
        }
        for c in range(NCORES)
    ]


def kernel(A_norm, X, W0, W1):
    A_norm = np.ascontiguousarray(A_norm, dtype=np.float32)
    X = np.ascontiguousarray(X, dtype=np.float32)
    W0 = np.ascontiguousarray(W0, dtype=np.float32)
    W1 = np.ascontiguousarray(W1, dtype=np.float32)

    if PRECISION not in _CACHED:
        _CACHED[PRECISION] = build_gcn()
    nc = _CACHED[PRECISION]

    in_maps = shard_inputs(A_norm, X, W0, W1)
    try:
        res = _exec_staged(nc, in_maps)
    except Exception:
        from concourse.bass_utils import run_bass_kernel_spmd

        res = run_bass_kernel_spmd(
            nc, in_maps, core_ids=list(range(NCORES))
        ).results
    return np.concatenate([res[c]["h_out"] for c in range(NCORES)], axis=0)


# revision 21
# speedup vs baseline: 1.0953x; 1.0953x over previous
"""2-layer dense GCN on 8 Trainium2 NeuronCores — fp8 residual + DoubleRow.

Reference computation (all fp32):
    H0 = relu((A_norm @ X) @ W0)
    H1 = relu((A_norm @ H0) @ W1)
A_norm: [16384, 16384] row-stochastic, X: [16384, 128], W0/W1: [128, 128].

Scheme: A_norm rows sum to exactly 1, so A = (1/N)*ones + R with R zero-mean
uniform. Only the residual is streamed, as e4m3 scaled to full range; the
rank-1 part is an exact per-feature bias (s*mu * colsum(H), with the TRUE
colsum for X host-side; for layer 1 the colsum is computed on-device from the
gathered e5m2 H via 32 width-512 ones-matmuls into a [1,512] PSUM bank — its
quantization noise averages down over 16384 nodes). Stationary X/H live in
e5m2. Aggregation runs in DoubleRow mode (2 contraction subtiles per PE pass).

Device structure (per core, 2048 output rows, 1D row shard):
  - layer 0 chunk-major: one 512-row output chunk per full-contraction pass;
    eviction: mt = (psum + bias)*(1/s) fp16; H = relu(mt.T @ W) written
    straight to e5m2; one SWDGE bounce write + AllGather doorbell per chunk.
  - exchange: FOUR chunked AllGathers (64 KiB e5m2 payloads). All stat1
    gather loads ride the gpsimd queue right behind their doorbell, so each
    fires the instant its AllGather lands and the HWDGE rings stay pure
    A-stream.
  - layer 1 quarter-major: pass q sweeps all 4 output chunks against
    stationary quarter q (gated only on AllGather q). The q3 A-block column
    plus (c2,q2),(c3,q2) stay RESIDENT in SBUF from layer 0 (12 MiB cache),
    so the post-AG3 tail needs no A DMA at all: stat1-q3 load + 64 DR
    matmuls + evictions (~30 us). Layer 1 streams only 10 of 16 blocks.

DMA routing: the A stream (26 blocks x 2 MiB) owns the two HWDGE rings
(sync/scalar) end to end; stationary X is ONE 2 MiB HWDGE load; everything
collective-gated is SWDGE so AllGather waits never block the A stream.

Dispatch: inputs are pre-staged onto all 8 cores (device_put + block) before
the single execution, so per-core start skew is not inflated by operand
transfer ordering.
"""

import sys
from contextlib import ExitStack

if "/opt/trn_rl_repo" not in sys.path:
    sys.path.insert(0, "/opt/trn_rl_repo")

import numpy as np

N_NODES = 16384
D = 128
NCORES = 8
ROWS = N_NODES // NCORES      # 2048
NCH = 4                       # output chunks per core (512 rows each)
IC = ROWS // NCH              # 512
NQ = 4                        # contraction quarters (= exchange chunks)
NT = 4                        # j-subtiles per (quarter, peer)

# layer-1 A-blocks resident in SBUF from layer 0: the whole q3 column, so
# the post-AG3 tail needs no A DMA at all. The rest of SBUF goes to stream
# depth (a_pool) — a deeper pool decouples the layer-1 prefetch from
# layer 0's PE-paced consumption (buffer recycling stalls).
CACHED = {(0, 3), (1, 3), (2, 3), (3, 3)}

PRECISION = "dr"  # tag for test.py compatibility
DEBUG = False     # adds intermediate dumps (d_h0, d_sig, d_bias) as outputs


def build_gcn():
    import concourse.bass as bass  # noqa: F401
    import concourse.tile as tile
    from concourse import bacc, mybir

    F32 = mybir.dt.float32
    F16 = mybir.dt.float16
    E4 = mybir.dt.float8e4
    E5 = mybir.dt.float8e5
    DR = mybir.MatmulPerfMode.DoubleRow
    relu = mybir.ActivationFunctionType.Relu
    add = mybir.AluOpType.add
    mult = mybir.AluOpType.mult

    nc = bacc.Bacc("TRN2", target_bir_lowering=False, num_devices=NCORES)

    # A residual, host pre-tiled into 2 MiB blocks (16 KiB partition lines):
    # block (c, q) is rows [(c*4+q)*128, +128); element (p, (r*4+t)*512 + cc)
    #   = s * R[myrows0 + c*512 + cc, r*2048 + q*512 + t*128 + p]
    a_in = nc.dram_tensor(
        "a0", [NCH * NQ * 128, NCORES * NT * IC], E4, kind="ExternalInput"
    )
    # X stationary as ONE contiguous [128, 16384] block:
    # x0[p, (r*16 + q*4 + t)*128 + dd] = X[r*2048 + q*512 + t*128 + p, dd]
    x_in = nc.dram_tensor("x0", [128, NCORES * NQ * NT * D], E5, kind="ExternalInput")
    w0 = nc.dram_tensor("w0", [D, D], F16, kind="ExternalInput")
    w1 = nc.dram_tensor("w1", [D, D], F16, kind="ExternalInput")
    b0 = nc.dram_tensor("b0", [D, 1], F32, kind="ExternalInput")   # s*mu*colsum(X)
    smu = nc.dram_tensor("smu", [D, 1], F32, kind="ExternalInput")  # s*mu
    is_in = nc.dram_tensor("is_", [D, 1], F32, kind="ExternalInput")  # 1/s
    h_out = nc.dram_tensor("h_out", [ROWS, D], F32, kind="ExternalOutput")
    if DEBUG:
        d_h0 = nc.dram_tensor("d_h0", [128, NCH * IC], E5, kind="ExternalOutput")
        d_sig = nc.dram_tensor("d_sig", [1, IC], F32, kind="ExternalOutput")
        d_bias = nc.dram_tensor("d_bias", [D, 1], F32, kind="ExternalOutput")

    with tile.TileContext(nc) as tc, ExitStack() as ctx:
        sb1 = ctx.enter_context(tc.tile_pool(name="sb1", bufs=1))
        stat0_pool = ctx.enter_context(tc.tile_pool(name="stat0", bufs=1))
        stat1_pool = ctx.enter_context(tc.tile_pool(name="stat1", bufs=NQ))
        a_pool = ctx.enter_context(tc.tile_pool(name="a", bufs=6))
        ac_pool = ctx.enter_context(tc.tile_pool(name="ac", bufs=len(CACHED)))
        m_pool = ctx.enter_context(tc.tile_pool(name="m", bufs=2))
        hc_pool = ctx.enter_context(tc.tile_pool(name="hc", bufs=2))
        h_pool = ctx.enter_context(tc.tile_pool(name="h", bufs=4))
        agg_pool = ctx.enter_context(tc.tile_pool(name="agg", bufs=4, space="PSUM"))
        sig_pool = ctx.enter_context(tc.tile_pool(name="sig", bufs=1, space="PSUM"))
        lin_pool = ctx.enter_context(tc.tile_pool(name="lin", bufs=2, space="PSUM"))
        sgt_pool = ctx.enter_context(tc.tile_pool(name="sgt", bufs=1, space="PSUM"))
        dram = ctx.enter_context(tc.tile_pool(name="dram", bufs=1, space="DRAM"))

        # small constants via SWDGE; stationary X as one HWDGE load on the
        # sync ring while the scalar ring starts the A stream
        w0_sb = sb1.tile([D, D], F16)
        nc.gpsimd.dma_start(out=w0_sb[:], in_=w0[:])
        w1_sb = sb1.tile([D, D], F16)
        nc.gpsimd.dma_start(out=w1_sb[:], in_=w1[:])
        b0_sb = sb1.tile([D, 1], F32)
        nc.gpsimd.dma_start(out=b0_sb[:], in_=b0[:])
        smu_sb = sb1.tile([D, 1], F32)
        nc.gpsimd.dma_start(out=smu_sb[:], in_=smu[:])
        is_sb = sb1.tile([D, 1], F32)
        nc.gpsimd.dma_start(out=is_sb[:], in_=is_in[:])
        ones8 = sb1.tile([D, 1], E5)
        nc.vector.memset(ones8[:], 1.0)
        one1 = sb1.tile([1, 1], F32)
        nc.vector.memset(one1[:], 1.0)

        # stat0 split in halves across both rings, after each ring's first
        # A-half, so neither ring's A stream is displaced by the full 2 MiB
        stat0 = stat0_pool.tile([128, NCORES * NQ * NT, D], E5, name="sx")
        HW = NCORES * NQ * NT // 2  # 64 subtiles = 8 KiB per partition line

        # exchange buffers: one bounce + gathered tensor per chunk
        h_tb = [dram.tile([128, IC], E5, name=f"h_tb{c}") for c in range(NCH)]
        h_ag = [
            dram.tile([NCORES, 128, IC], E5, addr_space="Shared", name=f"h_ag{c}")
            for c in range(NCH)
        ]
        stat1 = [
            stat1_pool.tile([128, NCORES * NT, D], E5, name=f"sh{q}", tag="s1")
            for q in range(NQ)
        ]



        a_cache = {}
        rings = [nc.scalar, nc.sync]
        AH = NCORES * NT // 2  # 16 subtiles = 8 KiB per partition line

        def a_load(c, q):
            """Each block split into peer-halves, one per ring: the block
            lands in ~6 us (ring cadence) instead of ~12, and peers 0-3's
            matmuls can start while peers 4-7 still stream."""
            if (c, q) in CACHED:
                at = ac_pool.tile(
                    [128, NCORES * NT, IC], E4, name=f"ac{c}{q}", tag="ac"
                )
                a_cache[(c, q)] = at
            else:
                at = a_pool.tile([128, NCORES * NT, IC], E4, name="at", tag="at")
            blk = c * NQ + q
            rows = a_in[blk * 128 : (blk + 1) * 128, :]
            rings[0].dma_start(out=at[:, 0:AH, :], in_=rows[:, 0 : AH * IC])
            rings[1].dma_start(out=at[:, AH:, :], in_=rows[:, AH * IC :])
            return at

        def sweep(agg, at, lhsT_fn, first, last):
            """16 DoubleRow matmuls over (peer r, subtile pair tp)."""
            for r in range(NCORES):
                for tp in range(0, NT, 2):
                    nc.tensor.matmul(
                        agg[:],
                        lhsT=lhsT_fn(r, tp),
                        rhs=at[:, r * NT + tp : r * NT + tp + 2, :],
                        start=first and r == 0 and tp == 0,
                        stop=last and r == NCORES - 1 and tp == NT - 2,
                        perf_mode=DR,
                    )

        # ---------------- layer 0 (chunk-major) ----------------
        # first A block ahead of stat0 on both rings: peer-0 matmuls can
        # start ~7 us in, and the c0->AllGather0 chain begins ASAP
        at00 = a_load(0, 0)
        rings[0].dma_start(out=stat0[:, 0:HW, :], in_=x_in[:, 0 : HW * D])
        rings[1].dma_start(out=stat0[:, HW:, :], in_=x_in[:, HW * D :])

        for c in range(NCH):
            agg = agg_pool.tile([128, IC], F32, name="ps", tag="ps")
            for q in range(NQ):
                at = at00 if (c == 0 and q == 0) else a_load(c, q)
                sweep(
                    agg,
                    at,
                    lambda r, tp, q=q: stat0[
                        :, r * NQ * NT + q * NT + tp : r * NQ * NT + q * NT + tp + 2, :
                    ],
                    first=q == 0,
                    last=q == NQ - 1,
                )
            mt = m_pool.tile([128, IC], F16, name="mt", tag="mt")
            nc.vector.tensor_scalar(
                out=mt[:], in0=agg[:], scalar1=b0_sb[:], scalar2=is_sb[:],
                op0=add, op1=mult,
            )
            hc = hc_pool.tile([128, IC], E5, name="hc", tag="hc")
            for t in range(NT):
                lp = lin_pool.tile([128, D], F32, name="lp", tag="lp")
                nc.tensor.matmul(
                    lp[:],
                    lhsT=mt[:, t * 128 : (t + 1) * 128],
                    rhs=w0_sb[:],
                    start=True,
                    stop=True,
                )
                ht = h_pool.tile([128, D], F32, name="ht", tag="ht")
                nc.scalar.activation(ht[:], lp[:], relu)
                nc.vector.tensor_copy(
                    out=hc[:, t * 128 : (t + 1) * 128], in_=ht[:]
                )
            # bounce on a HWDGE ring (a SWDGE bounce's completion receipt is
            # delayed ~20us while an AllGather wave is active, which stalls
            # the next doorbell); doorbell + gather load stay on gpsimd so
            # the stat1 load fires the moment AllGather c completes
            if DEBUG:
                nc.gpsimd.dma_start(out=d_h0[:, c * IC : (c + 1) * IC], in_=hc[:])
            rings[c % 2].dma_start(out=h_tb[c][:], in_=hc[:])
            nc.gpsimd.collective_compute(
                "AllGather",
                mybir.AluOpType.bypass,
                replica_groups=[list(range(NCORES))],
                ins=[h_tb[c][:]],
                outs=[h_ag[c][:]],
            )
            # single strided load of the whole gathered quarter: [8,128,IC]
            # viewed as [128, 8, IC] — one DMA instead of 8, so the next
            # chunk's doorbell is not stuck behind a long SWDGE queue
            nc.gpsimd.dma_start(
                out=stat1[c][:], in_=h_ag[c][:].transpose([1, 0, 2])
            )

        # ---------------- layer 1 (quarter-major) ----------------
        aggs = [
            agg_pool.tile([128, IC], F32, name=f"ps{c}", tag="ps") for c in range(NCH)
        ]
        sig = sig_pool.tile([1, IC], F32, name="sg")

        def sig_mms(q):
            # colsum of gathered H, one width-512 matmul per peer
            for r in range(NCORES):
                nc.tensor.matmul(
                    sig[:],
                    lhsT=ones8[:],
                    rhs=stat1[q][:, r * NT : (r + 1) * NT, :],
                    start=q == 0 and r == 0,
                    stop=q == NQ - 1 and r == NCORES - 1,
                )

        def evict1(c, bias1):
            mt = m_pool.tile([128, IC], F16, name="mt", tag="mt")
            nc.vector.tensor_scalar(
                out=mt[:], in0=aggs[c][:], scalar1=bias1[:], scalar2=is_sb[:],
                op0=add, op1=mult,
            )
            for t in range(NT):
                lp = lin_pool.tile([128, D], F32, name="lp", tag="lp")
                nc.tensor.matmul(
                    lp[:],
                    lhsT=mt[:, t * 128 : (t + 1) * 128],
                    rhs=w1_sb[:],
                    start=True,
                    stop=True,
                )
                ht = h_pool.tile([128, D], F32, name="ht", tag="ht")
                nc.scalar.activation(ht[:], lp[:], relu)
                eng = rings[(c * NT + t) % 2]
                eng.dma_start(
                    out=h_out[c * IC + t * 128 : c * IC + (t + 1) * 128, :],
                    in_=ht[:],
                )

        bias1 = None
        for q in range(NQ):
            ats = []
            for c in range(NCH):
                if (c, q) in CACHED:
                    ats.append(a_cache[(c, q)])
                else:
                    ats.append(a_load(c, q))
            if q == NQ - 1:
                sig_mms(q)  # before the sweeps: closes the sigma accumulation
            for c in range(NCH):
                sweep(
                    aggs[c],
                    ats[c],
                    lambda r, tp, q=q: stat1[q][:, r * NT + tp : r * NT + tp + 2, :],
                    first=q == 0,
                    last=q == NQ - 1,
                )
                if q == NQ - 1:
                    if c == 0:
                        # sigma -> bias1: reduce 4 subtile groups, transpose
                        # [1,128] -> [128,1] via a 1-partition matmul, * s*mu
                        sigsb = sb1.tile([1, IC], F32, name="sigsb")
                        nc.vector.tensor_copy(out=sigsb[:], in_=sig[:])
                        p01 = sb1.tile([1, D], F32, name="p01")
                        nc.vector.tensor_tensor(
                            out=p01[:], in0=sigsb[:, 0:128], in1=sigsb[:, 128:256],
                            op=add,
                        )
                        p23 = sb1.tile([1, D], F32, name="p23")
                        nc.vector.tensor_tensor(
                            out=p23[:], in0=sigsb[:, 256:384], in1=sigsb[:, 384:512],
                            op=add,
                        )
                        s4 = sb1.tile([1, D], F32, name="s4")
                        nc.vector.tensor_tensor(
                            out=s4[:], in0=p01[:], in1=p23[:], op=add
                        )
                        sgt = sgt_pool.tile([D, 1], F32, name="sgt")
                        nc.tensor.matmul(
                            sgt[:], lhsT=s4[:], rhs=one1[:], start=True, stop=True
                        )
                        bias1 = sb1.tile([D, 1], F32, name="bias1")
                        nc.vector.tensor_tensor(
                            out=bias1[:], in0=sgt[:], in1=smu_sb[:], op=mult
                        )
                        if DEBUG:
                            nc.gpsimd.dma_start(out=d_sig[:], in_=sigsb[:])
                            nc.gpsimd.dma_start(out=d_bias[:], in_=bias1[:])
                    evict1(c, bias1)
            if q < NQ - 1:
                sig_mms(q)  # sigma contributions for this quarter

    nc.finalize()
    return nc


def shard_inputs(A_norm, X, W0, W1, precision=None):
    """Host-side prep. Returns per-core input maps (complete, incl. weights)."""
    import ml_dtypes

    e4m3 = ml_dtypes.float8_e4m3
    e5m2 = ml_dtypes.float8_e5m2
    N = N_NODES
    mu = np.float32(1.0 / N)

    R = A_norm.astype(np.float32) - mu
    s = float(ml_dtypes.finfo(e4m3).max) / float(np.abs(R).max())
    Rq = (R * np.float32(s)).astype(e4m3)
    del R

    # x0[p, (r*16 + q*4 + t)*128 + dd] = X[r*2048 + (q*4+t)*128 + p, dd]
    x0 = np.ascontiguousarray(
        X.astype(e5m2)
        .reshape(NCORES, NQ * NT, 128, D)
        .transpose(2, 0, 1, 3)
        .reshape(128, NCORES * NQ * NT * D)
    )

    smu_v = np.float32(s * mu)
    smu = np.full((D, 1), smu_v, dtype=np.float32)
    # TRUE colsum of X (float64) — kills the rank-1 projection of X's
    # quantization noise
    b0 = (
        np.float64(s) * np.float64(mu) * X.astype(np.float64).sum(axis=0)
    ).astype(np.float32).reshape(D, 1)
    is_ = np.full((D, 1), np.float32(1.0) / np.float32(s), dtype=np.float32)
    w0 = W0.astype(np.float16)
    w1 = W1.astype(np.float16)

    in_maps = []
    for core in range(NCORES):
        Rt = Rq[core * ROWS : (core + 1) * ROWS, :].T  # [16384 nodes, 2048]
        # [r, q, t, p, c, cc] -> rows (c*4+q)*128+p, cols (r*4+t)*512+cc
        a0 = np.ascontiguousarray(
            Rt.reshape(NCORES, NQ, NT, 128, NCH, IC)
            .transpose(4, 1, 3, 0, 2, 5)
            .reshape(NCH * NQ * 128, NCORES * NT * IC)
        )
        in_maps.append(
            {"a0": a0, "x0": x0, "w0": w0, "w1": w1, "b0": b0, "smu": smu,
             "is_": is_}
        )
    return in_maps


_CACHED = {}
_EXEC = {}


def _exec_staged(nc, in_maps):
    """Run the prebuilt module via PJRT with inputs pre-staged (device_put +
    block_until_ready) so all 8 cores launch with operands already resident —
    avoids per-core start skew from operand transfer ordering."""
    import jax
    from jax.experimental.shard_map import shard_map
    from jax.sharding import Mesh, NamedSharding, PartitionSpec

    from concourse import bass2jax, mybir

    key = id(nc)
    if key not in _EXEC:
        bass2jax.install_neuronx_cc_hook()
        partition_name = (
            nc.partition_id_tensor.name if nc.partition_id_tensor else None
        )
        in_names, out_names, out_avals = [], [], []
        for alloc in nc.m.functions[0].allocations:
            if not isinstance(alloc, mybir.MemoryLocationSet):
                continue
            name = alloc.memorylocations[0].name
            if alloc.kind == "ExternalInput":
                if name != partition_name:
                    in_names.append(name)
            elif alloc.kind == "ExternalOutput":
                out_names.append(name)
                shape = tuple(alloc.tensor_shape)
                dtype = mybir.dt.np(alloc.dtype)
                out_avals.append(jax.core.ShapedArray(shape, dtype))
        n_params = len(in_names)
        all_in = list(in_names) + list(out_names)
        if partition_name is not None:
            all_in.append(partition_name)

        def _body(*args):
            operands = list(args)
            if partition_name is not None:
                operands.append(bass2jax.partition_id_tensor())
            outs = bass2jax._bass_exec_p.bind(
                *operands,
                out_avals=tuple(out_avals),
                in_names=tuple(all_in),
                out_names=tuple(out_names),
                lowering_input_output_aliases=(),
                sim_require_finite=True,
                sim_require_nnan=True,
                nc=nc,
            )
            return tuple(outs)

        devices = jax.devices()[:NCORES]
        mesh = Mesh(np.asarray(devices), ("core",))
        n_outs = len(out_avals)
        sharded = jax.jit(
            shard_map(
                _body,
                mesh=mesh,
                in_specs=(PartitionSpec("core"),) * (n_params + n_outs),
                out_specs=(PartitionSpec("core"),) * n_outs,
                check_rep=False,
            ),
            donate_argnums=tuple(range(n_params, n_params + n_outs)),
            keep_unused=True,
        )
        _EXEC[key] = (sharded, in_names, out_names, out_avals, mesh)
    sharded, in_names, out_names, out_avals, mesh = _EXEC[key]

    sh = NamedSharding(mesh, PartitionSpec("core"))
    concat_in = [
        np.concatenate([np.asarray(in_maps[c][nm]) for c in range(NCORES)], axis=0)
        for nm in in_names
    ]
    concat_zeros = [
        np.zeros((NCORES * a.shape[0], *a.shape[1:]), a.dtype) for a in out_avals
    ]
    import jax

    staged = [jax.device_put(a, sh) for a in concat_in + concat_zeros]
    for a in staged:
        a.block_until_ready()
    out_arrs = sharded(*staged)
    return [
        {
            nm: np.asarray(out_arrs[i]).reshape(NCORES, *out_avals[i].shape)[c]
            for i, nm in enumerate(out_names)
        }
        for c in range(NCORES)
    ]


def kernel(A_norm, X, W0, W1):
    A_norm = np.ascontiguousarray(A_norm, dtype=np.float32)
    X = np.ascontiguousarray(X, dtype=np.float32)
    W0 = np.ascontiguousarray(W0, dtype=np.float32)
    W1 = np.ascontiguousarray(W1, dtype=np.float32)

    if PRECISION not in _CACHED:
        _CACHED[PRECISION] = build_gcn()
    nc = _CACHED[PRECISION]

    in_maps = shard_inputs(A_norm, X, W0, W1)
    try:
        res = _exec_staged(nc, in_maps)
    except Exception:
        from concourse.bass_utils import run_bass_kernel_spmd

        res = run_bass_kernel_spmd(
            nc, in_maps, core_ids=list(range(NCORES))
        ).results
    return np.concatenate([res[c]["h_out"] for c in range(NCORES)], axis=0)
